# revision 7
# baseline (speedup 1.0000x reference)
"""Trainium2 Bass kernel for nn_DiffusionGraphConv (gnn_message_passing).

Reference computation (B=64, N=1024, D=128=64+64, O=128, 2 supports,
2 diffusion steps):
    x0 = concat(inputs, state)                      # [B, N, D]
    y1 = S0 x0 ; z2 = S0 y1 ; y3 = S1 y1 ; z4 = S1 y3
    xs = [x0, y1, 2 z2 - x0, y3, 2 z4 - y1]
    out = concat_d(xs) @ W + bias                   # [B*N, O]

Polynomial refactor: fold the +-/2x into weight blocks
    Wa = W0 - W2, Wb = W1 - W4, Wc = 2 W2, Wd = W3, We = 2 W4
then commute the (cheap, K=128) feature projections with the node-space
supports and hoist the batch-independent support polynomials:
    M1 = S0, M2 = S0^2, M3 = S1 S0, M4 = S1^2 S0      (precomputed once)
    out = (x0 Wa + bias) + sum_k M_k (x0 W'_k),  W' = [Wb, Wc, Wd, We]

Sharding: data-parallel over batch, 8 batches per NeuronCore; supports,
their polynomials and the weights stay SBUF-resident. Per-core schedule:
    pre:   UT = T1 T1 ; A2T = T0 T0 ; A3T = T0 T1 ; A4T = T0 UT
           (T0 = S0^T, T1 = S1^T; AkT = M_k^T, the lhsT the apply needs)
    per rep:
      Q:     Qa..Qe = x0 W'_k per (nt, h): one stationary x0t slice feeds
             5 matmuls into 5 PSUM banks (Qa drains fused with + bias)
      apply: out[it, f] = PSUM( sum_{k,jt} AkT MM Qk ) + Qab   (32 MMs of
             512 cols per PSUM group; DVE add + DMA out)
All accumulation fp32 in PSUM; operands bf16.
"""
import sys

if "/opt/trn_rl_repo" not in sys.path:
    sys.path.insert(0, "/opt/trn_rl_repo")

import numpy as np
import ml_dtypes

import concourse.bass as bass
import concourse.mybir as mybir
from concourse import bacc, tile
from concourse.bass_utils import run_bass_kernel_spmd

N_CORES = 8
B = 64
BL = B // N_CORES          # local batches per core
N = 1024                   # nodes
D = 128                    # input_size (64 input + 64 hidden)
O = 128                    # output_size
NT = N // 128              # node partition tiles
BF16 = mybir.dt.bfloat16
F32 = mybir.dt.float32

_CACHE = {}


def _build(reps=1):
    nc = bacc.Bacc("TRN2", target_bir_lowering=False, debug=False,
                   num_devices=N_CORES)
    s0t_d = nc.dram_tensor("s0t", [N, N], BF16, kind="ExternalInput").ap()
    s0n_d = nc.dram_tensor("s0n", [N, N], BF16, kind="ExternalInput").ap()
    s1t_d = nc.dram_tensor("s1t", [N, N], BF16, kind="ExternalInput").ap()
    s1n_d = nc.dram_tensor("s1n", [N, N], BF16, kind="ExternalInput").ap()
    x0t_d = nc.dram_tensor("x0t", [BL * D, N], BF16, kind="ExternalInput").ap()
    wf_d = nc.dram_tensor("wf", [5 * D, O], BF16, kind="ExternalInput").ap()
    bias_d = nc.dram_tensor("biasb", [128, 512], F32, kind="ExternalInput").ap()
    out_d = nc.dram_tensor("out", [N, BL, O], BF16, kind="ExternalOutput").ap()

    with tile.TileContext(nc) as tc:
        with (
            tc.tile_pool(name="main", bufs=1) as mp,
            tc.tile_pool(name="outp", bufs=4) as op,
            tc.tile_pool(name="psb", bufs=8, space="PSUM") as pb,
        ):
            # ---- persistent SBUF residents ----
            # DMA emission order = consumption order (precompute first).
            s1n = []   # buffers later reused as Qb
            s1t = []
            s0n = []   # later reused as Qc
            s0t = []
            for j in range(NT):
                t = mp.tile([128, N], BF16, tag=f"qb{j}", name=f"s1n{j}")
                nc.sync.dma_start(out=t[:], in_=s1n_d[j * 128:(j + 1) * 128, :])
                s1n.append(t)
                t = mp.tile([128, N], BF16, tag=f"s1t{j}", name=f"s1t{j}")
                nc.sync.dma_start(out=t[:], in_=s1t_d[j * 128:(j + 1) * 128, :])
                s1t.append(t)
            for j in range(NT):
                t = mp.tile([128, N], BF16, tag=f"qc{j}", name=f"s0n{j}")
                nc.sync.dma_start(out=t[:], in_=s0n_d[j * 128:(j + 1) * 128, :])
                s0n.append(t)
                t = mp.tile([128, N], BF16, tag=f"s0t{j}", name=f"s0t{j}")
                nc.sync.dma_start(out=t[:], in_=s0t_d[j * 128:(j + 1) * 128, :])
                s0t.append(t)
            x0t0 = []
            for b in range(BL):
                t = mp.tile([128, N], BF16, tag=f"x0t{b}", name=f"x0t{b}_p")
                nc.sync.dma_start(out=t[:], in_=x0t_d[b * 128:(b + 1) * 128, :])
                x0t0.append(t)
            w = []
            for k in range(5):
                t = mp.tile([128, O], BF16, tag=f"w{k}", name=f"w{k}")
                nc.sync.dma_start(out=t[:], in_=wf_d[k * 128:(k + 1) * 128, :])
                w.append(t)
            bias_t = mp.tile([128, 512], F32, tag="bias")
            nc.sync.dma_start(out=bias_t[:], in_=bias_d[:])

            ci = 0

            def pcopy(dst, src):
                # alternate DVE / ACT for PSUM->SBUF moves
                nonlocal ci
                if ci % 2 == 0:
                    nc.vector.tensor_copy(dst, src)
                else:
                    nc.scalar.copy(dst, src)
                ci += 1

            # ---- precompute support polynomials (batch-independent) ----
            # UT = T1 T1, A2T = T0 T0, A3T = T0 T1, A4T = T0 UT
            ut = [mp.tile([128, N], BF16, tag=f"qd{j}", name=f"ut{j}")
                  for j in range(NT)]
            a2t = [mp.tile([128, N], BF16, tag=f"a2t{j}", name=f"a2t{j}")
                   for j in range(NT)]
            a3t = [mp.tile([128, N], BF16, tag=f"a3t{j}", name=f"a3t{j}")
                   for j in range(NT)]
            a4t = [mp.tile([128, N], BF16, tag=f"a4t{j}", name=f"a4t{j}")
                   for j in range(NT)]
            for dst, lhs, rhs in ((ut, s1n, s1t), (a2t, s0n, s0t),
                                  (a3t, s0n, s1t), (a4t, s0n, ut)):
                for it in range(NT):
                    for f in range(2):
                        ps = pb.tile([128, 512], F32, tag="big")
                        for jt in range(NT):
                            nc.tensor.matmul(
                                ps[:],
                                lhs[jt][:, it * 128:(it + 1) * 128],
                                rhs[jt][:, f * 512:(f + 1) * 512],
                                start=(jt == 0), stop=(jt == NT - 1),
                            )
                        pcopy(dst[it][:, f * 512:(f + 1) * 512], ps[:])

            mats = [s0t, a2t, a3t, a4t]

            for rep in range(reps):
                # ---- x0 T-layout reload (rep 0 preloaded); its last reader
                # is this rep's Q phase, so rep r+1's DMA hides under rep
                # r's apply phase.
                if rep == 0:
                    x0t = x0t0
                else:
                    x0t = []
                    for b in range(BL):
                        t = mp.tile([128, N], BF16, tag=f"x0t{b}",
                                    name=f"x0t{b}_{rep}")
                        nc.sync.dma_start(
                            out=t[:], in_=x0t_d[b * 128:(b + 1) * 128, :])
                        x0t.append(t)

                qb = [mp.tile([128, BL * O], BF16, tag=f"qb{j}",
                              name=f"qb{j}_{rep}") for j in range(NT)]
                qc = [mp.tile([128, BL * O], BF16, tag=f"qc{j}",
                              name=f"qc{j}_{rep}") for j in range(NT)]
                qd = [mp.tile([128, BL * O], BF16, tag=f"qd{j}",
                              name=f"qd{j}_{rep}") for j in range(NT)]
                qe = [mp.tile([128, BL * O], BF16, tag=f"qe{j}",
                              name=f"qe{j}_{rep}") for j in range(NT)]
                qab = [mp.tile([128, BL * O], F32, tag=f"qa{j}",
                               name=f"qa{j}_{rep}") for j in range(NT)]
                qs = [qab, qb, qc, qd, qe]

                # ---- Q phase: Qk = x0 W'_k, F-layout [n, (b,o)] ----
                # One stationary x0t slice feeds 5 matmuls (one per weight
                # block) into 5 PSUM banks; the bank ring (8) lets group
                # g+1 fill while group g drains.
                for nt in range(NT):
                    for h in range(2):
                        ps5 = [pb.tile([128, 512], F32, tag="big",
                                       name=f"q{k}_{rep}_{nt}_{h}")
                               for k in range(5)]
                        for q, bb in enumerate(range(4 * h, 4 * h + 4)):
                            stat = x0t[bb][:, nt * 128:(nt + 1) * 128]
                            for k in range(5):
                                nc.tensor.matmul(
                                    ps5[k][:, q * 128:(q + 1) * 128],
                                    stat, w[k][:],
                                    start=True, stop=True)
                        nc.vector.tensor_add(
                            qab[nt][:, h * 512:(h + 1) * 512], ps5[0][:],
                            bias_t[:])
                        for k in range(1, 5):
                            pcopy(qs[k][nt][:, h * 512:(h + 1) * 512],
                                  ps5[k][:])

                # ---- apply: out[it, f] = sum_k M_k Qk + Qab ----
                for it in range(NT):
                    for f in range(2):
                        ps = pb.tile([128, 512], F32, tag="big",
                                     name=f"fin_{rep}_{it}_{f}")
                        first = True
                        for k in range(4):
                            akt = mats[k]
                            qk = qs[k + 1]
                            for jt in range(NT):
                                nc.tensor.matmul(
                                    ps[:],
                                    akt[jt][:, it * 128:(it + 1) * 128],
                                    qk[jt][:, f * 512:(f + 1) * 512],
                                    start=first,
                                    stop=(k == 3 and jt == NT - 1),
                                )
                                first = False
                        ot = op.tile([128, 512], BF16, tag="out")
                        nc.vector.tensor_add(
                            ot[:], ps[:],
                            qab[it][:, f * 512:(f + 1) * 512])
                        nc.sync.dma_start(
                            out=out_d[it * 128:(it + 1) * 128,
                                      4 * f:4 * f + 4, :],
                            in_=ot[:])
    nc.compile()
    return nc


def _prep_inputs(supports, inputs, state, weight, biases):
    supports = np.asarray(supports, dtype=np.float32)
    inputs = np.asarray(inputs, dtype=np.float32)
    state = np.asarray(state, dtype=np.float32)
    weight = np.asarray(weight, dtype=np.float32)
    biases = np.asarray(biases, dtype=np.float32)

    s0n = supports[0].astype(ml_dtypes.bfloat16)
    s0t = np.ascontiguousarray(supports[0].T).astype(ml_dtypes.bfloat16)
    s1n = supports[1].astype(ml_dtypes.bfloat16)
    s1t = np.ascontiguousarray(supports[1].T).astype(ml_dtypes.bfloat16)

    x0 = np.concatenate(
        [inputs.reshape(B, N, D // 2), state.reshape(B, N, D // 2)], axis=2)
    x0t = np.ascontiguousarray(x0.transpose(0, 2, 1))      # [B, D, N]
    x0t_bf = x0t.astype(ml_dtypes.bfloat16)

    W = weight.reshape(5, D, O)
    wf = np.concatenate([
        W[0] - W[2],        # Wa
        W[1] - W[4],        # Wb
        2.0 * W[2],         # Wc
        W[3],               # Wd
        2.0 * W[4],         # We
    ], axis=0).astype(ml_dtypes.bfloat16)

    biasb = np.ascontiguousarray(np.tile(biases[None, :], (128, 4)))

    in_maps = []
    for c in range(N_CORES):
        bsl = slice(c * BL, (c + 1) * BL)
        in_maps.append({
            "s0t": s0t,
            "s0n": s0n,
            "s1t": s1t,
            "s1n": s1n,
            "x0t": np.ascontiguousarray(x0t_bf[bsl]).reshape(BL * D, N),
            "wf": wf,
            "biasb": biasb,
        })
    return in_maps


def _get_runner(reps=1):
    """Build the jitted SPMD executor once (mirrors
    bass2jax.run_bass_via_pjrt) so repeated calls don't re-trace."""
    if ("runner", reps) in _CACHE:
        return _CACHE[("runner", reps)]
    import jax
    from jax.sharding import Mesh, PartitionSpec, NamedSharding
    from concourse import bass2jax
    import concourse.mybir as mb

    try:
        jax.config.update("jax_compilation_cache_dir", "/tmp/jax_cache")
        jax.config.update("jax_persistent_cache_min_compile_time_secs", 1.0)
    except Exception:
        pass

    if ("nc", reps) not in _CACHE:
        _CACHE[("nc", reps)] = _build(reps=reps)
    nc = _CACHE[("nc", reps)]
    bass2jax.install_neuronx_cc_hook()

    part_name = nc.partition_id_tensor.name if nc.partition_id_tensor else None
    in_names, out_names, out_avals, zero_outs = [], [], [], []
    for alloc in nc.m.functions[0].allocations:
        if not isinstance(alloc, mb.MemoryLocationSet):
            continue
        name = alloc.memorylocations[0].name
        if alloc.kind == "ExternalInput":
            if name != part_name:
                in_names.append(name)
        elif alloc.kind == "ExternalOutput":
            out_names.append(name)
            shape = tuple(alloc.tensor_shape)
            dtype = mb.dt.np(alloc.dtype)
            out_avals.append(jax.core.ShapedArray(shape, dtype))
            zero_outs.append(np.zeros(shape, dtype))
    n_params = len(in_names)
    all_names = in_names + out_names
    if part_name is not None:
        all_names = all_names + [part_name]

    def _body(*args):
        operands = list(args)
        if part_name is not None:
            operands.append(bass2jax.partition_id_tensor())
        outs = bass2jax._bass_exec_p.bind(
            *operands,
            out_avals=tuple(out_avals),
            in_names=tuple(all_names),
            out_names=tuple(out_names),
            lowering_input_output_aliases=(),
            sim_require_finite=True,
            sim_require_nnan=True,
            nc=nc,
        )
        return tuple(outs)

    devices = jax.devices()[:N_CORES]
    mesh = Mesh(np.asarray(devices), ("core",))
    from jax.experimental.shard_map import shard_map
    n_outs = len(out_names)
    donate = tuple(range(n_params, n_params + n_outs))
    sharded = jax.jit(
        shard_map(_body, mesh=mesh,
                  in_specs=(PartitionSpec("core"),) * (n_params + n_outs),
                  out_specs=(PartitionSpec("core"),) * n_outs,
                  check_rep=False),
        donate_argnums=donate, keep_unused=True)
    sh = NamedSharding(mesh, PartitionSpec("core"))

    runner = {
        "fn": sharded, "in_names": in_names, "out_names": out_names,
        "zero_outs": zero_outs, "sharding": sh, "mesh": mesh,
    }
    _CACHE[("runner", reps)] = runner
    return runner


def _run(in_maps, device_inputs=None, reps=1):
    """Execute on the 8 cores; returns list of per-core output dicts."""
    import jax
    r = _get_runner(reps)
    if device_inputs is None:
        device_inputs = _put_inputs(in_maps, reps)
    zeros = [
        jax.device_put(
            np.zeros((N_CORES * z.shape[0], *z.shape[1:]), z.dtype),
            r["sharding"])
        for z in r["zero_outs"]
    ]
    out_arrs = r["fn"](*device_inputs, *zeros)
    outs = [np.asarray(a) for a in out_arrs]
    return [
        {name: outs[i].reshape(N_CORES, *r["zero_outs"][i].shape)[c]
         for i, name in enumerate(r["out_names"])}
        for c in range(N_CORES)
    ]


def _put_inputs(in_maps, reps=1):
    import jax
    r = _get_runner(reps)
    return [
        jax.device_put(
            np.concatenate([np.asarray(in_maps[c][n]) for c in range(N_CORES)],
                           axis=0), r["sharding"])
        for n in r["in_names"]
    ]


def kernel(supports, inputs, state, weight, biases, output_size=O, **_):
    assert int(output_size) == O
    in_maps = _prep_inputs(supports, inputs, state, weight, biases)
    res = _run(in_maps)
    # per-core out: [N, BL, O] -> full [B, N*O]
    outs = np.stack([res[c]["out"] for c in range(N_CORES)]).astype(np.float32)
    out = outs.transpose(0, 2, 1, 3).reshape(B, N * O)
    return np.ascontiguousarray(out)


if __name__ == "__main__":
    rng = np.random.default_rng(0)
    sup = rng.standard_normal((2, N, N)).astype(np.float32) / np.sqrt(N)
    inp = rng.standard_normal((B, N * 64)).astype(np.float32)
    st = rng.standard_normal((B, N * 64)).astype(np.float32)
    wt = rng.standard_normal((5 * D, O)).astype(np.float32) * 0.05
    bs = np.zeros((O,), np.float32)
    out = kernel(sup, inp, st, wt, bs, O)
    print("out", out.shape, out.dtype, float(np.abs(out).max()))
